# revision 56
# baseline (speedup 1.0000x reference)
"""Channel-attention (XCA-style) kernel for TRN2, 8 NeuronCores, data-parallel
over batch (1 image per core).

Per image:
  q  = conv3x3(y, Wq')  via fp8 DoubleRow matmuls (Wq' scaled x64; q is
       L2-normalized later so the scale cancels)
  kv1 = conv1x1(x) fp16, output channel order [v(192); k(192)]:
       ct0 = v 0:128, ct1 = v 128:192 | k 0:64, ct2 = k 64:192
  k  = dw3x3(k-part) via fp8 DoubleRow diag matmuls on PE (scaled x64,
       cancels in normalization)
  v  = dw3x3(v-part) on DVE/Pool (fp16), written directly into persistent
       SBUF tiles vA (v 0:128) / vB (v 128:192)
  Norms |q|,|k| from fp8 Gram diagonals (qq / kk) + qk Gram, all DoubleRow.
  A = softmax(G*t/(|q||k|)); CT = A @ P^T, shuffled to CTa/CTb via
  SBUF->SBUF DMA; out = (CTa^T @ vA + CTb^T @ vB) per tile.
"""
import numpy as np
import ml_dtypes

import concourse.bass as bass
import concourse.bacc as bacc
import concourse.mybir as mybir
import concourse.tile as tile
from concourse.masks import make_identity

F32 = mybir.dt.float32
FP16 = mybir.dt.float16
FP8 = mybir.dt.float8e4
DR = mybir.MatmulPerfMode.DoubleRow
WQ_SCALE = 64.0
WK_SCALE = 64.0

B, C, H, W = 8, 192, 128, 128
HEADS = 8
CH = C // HEADS            # 24
N = H * W                  # 16384
WP = W + 4                 # padded row stride for y (132)
HP = H + 2                 # padded rows (130)
NG = 8                     # row groups
GR = H // NG               # rows per group (16)
RT = 4                     # rows per conv tile
NT = H // RT               # conv tiles (32)
RS = W + 2                 # guard-column row stride (130)
GL = 2 + 18 * RS           # guard-layout tile length (2342)

CC = [(0, 96), (96, 96)]   # head-aligned channel chunking (4 heads each)

# DW tap order: tap = (dy+1)*3 + (dx+1); guard-layout offset of tap
TAPS = [(dy, dx) for dy in (-1, 0, 1) for dx in (-1, 0, 1)]
# DoubleRow ktile-pair stride must differ from the row (130) and column (1)
# strides of the rhs AP; these pairings give deltas of 2 and 260.
KD_PAIRS = [(0, 2), (3, 5), (6, 8), (1, 7), (4, None)]


def tap_off(t):
    dy, dx = TAPS[t]
    return 2 + (1 + dy) * RS + dx


USE_POOL = False


def build(repeat=1, use_for_i=False, parts=("q", "kv", "dw", "gram", "attn",
                                            "fin")):
    nc = bacc.Bacc()
    tok_in = nc.dram_tensor("tok_in", [128, 16], F32, kind="ExternalInput")
    # y prepacked host-side into the per-group DR subtile layout:
    # group g occupies rows [g*288, (g+1)*288): A=128, B=96, C=64 rows,
    # each row = 2 ktiles * 18 rows * WP fp8 — every DMA is one
    # contiguous DRAM block (descriptor-cheap).
    d_y = nc.dram_tensor("ypad", [NG * 288, 2 * 18 * WP], FP8,
                         kind="ExternalInput")
    # x tile-pair-major [t2, c, 1024] so per-pair chunk loads are contiguous
    d_x = nc.dram_tensor("x16", [(NT // 2) * C, 1024], FP16,
                         kind="ExternalInput")
    d_wq = nc.dram_tensor("wq", [3 * C, 3 * C], FP8, kind="ExternalInput")
    d_wkv = nc.dram_tensor("wkv", [C, 2 * C], FP16, kind="ExternalInput")
    d_wdv = nc.dram_tensor("wdv", [2 * 128, 16], F32, kind="ExternalInput")
    d_wdk1 = nc.dram_tensor("wdk1", [128, 10 * 64], FP8, kind="ExternalInput")
    d_wdk2 = nc.dram_tensor("wdk2", [128, 10 * 128], FP8, kind="ExternalInput")
    d_wp = nc.dram_tensor("wproj", [C, C], FP16, kind="ExternalInput")
    d_tv = nc.dram_tensor("tvec", [C, 16], F32, kind="ExternalInput")
    d_msk = nc.dram_tensor("smask", [C, C], F32, kind="ExternalInput")
    # out tile-pair-major [t2, c, 1024]; host transposes back
    d_out = nc.dram_tensor("out", [(NT // 2) * C, 1024], F32,
                           kind="ExternalOutput")
    d_tok = nc.dram_tensor("tok_out", [128, 16], F32, kind="ExternalOutput")

    with tile.TileContext(nc) as tc:
        with (
            tc.tile_pool(name="wp", bufs=1) as wp,
            tc.tile_pool(name="io", bufs=2) as io,
            tc.tile_pool(name="qt", bufs=1) as qtp,
            tc.tile_pool(name="dw", bufs=2) as dwp,
            tc.tile_pool(name="sm", bufs=1) as sm,
            tc.tile_pool(name="ps", bufs=1, space="PSUM") as ps,
        ):
            tki = sm.tile([128, 16], F32)
            nc.sync.dma_start(tki, tok_in[:, :])

            # ---------------- weights ----------------
            QSUB = [(128, (0, 128), (128, 256)),
                    (96, (256, 352), (352, 448)),
                    (64, (448, 512), (512, 576))]
            t_wq8 = []
            for i, (pn, (a0, a1), (b0, b1)) in enumerate(QSUB):
                w8 = wp.tile([pn, 2 * 3 * C], FP8, name=f"wq8_{i}")
                nc.sync.dma_start(w8[:, 0:3 * C], d_wq[a0:a1, :])
                nc.sync.dma_start(w8[:, 3 * C:6 * C], d_wq[b0:b1, :])
                t_wq8.append(w8)
            t_wkv = {}
            t_wp_ = {}
            t_tv = {}
            t_msk = {}
            for c0, cn in CC:
                t_wkv[c0] = wp.tile([cn, 2 * C], FP16, name=f"wkv{c0}")
                nc.sync.dma_start(t_wkv[c0], d_wkv[c0:c0 + cn, :])
                t_wp_[c0] = wp.tile([cn, C], FP16, name=f"wp{c0}")
                nc.sync.dma_start(t_wp_[c0], d_wp[c0:c0 + cn, :])
                t_tv[c0] = wp.tile([cn, 16], F32, name=f"tv{c0}")
                nc.sync.dma_start(t_tv[c0], d_tv[c0:c0 + cn, :])
                t_msk[c0] = wp.tile([cn, C], F32, name=f"msk{c0}")
                nc.sync.dma_start(t_msk[c0], d_msk[c0:c0 + cn, :])
            t_wdv = wp.tile([128, 32], F32, name="wdv")
            nc.sync.dma_start(t_wdv[:, 0:16], d_wdv[0:128, :])
            nc.sync.dma_start(t_wdv[:, 16:32], d_wdv[128:256, :])
            t_wdk1 = wp.tile([128, 10 * 64], FP8, name="wdk1")
            nc.sync.dma_start(t_wdk1, d_wdk1[:, :])
            t_wdk2 = wp.tile([128, 10 * 128], FP8, name="wdk2")
            nc.sync.dma_start(t_wdk2, d_wdk2[:, :])
            id32 = wp.tile([128, 128], F32, name="id32")
            make_identity(nc, id32)
            id16 = wp.tile([128, 128], FP16, name="id16")
            make_identity(nc, id16)
            ones1 = wp.tile([1, C], F32, name="ones1")
            nc.vector.memset(ones1, 1.0)

            # persistent v tiles
            vA = wp.tile([128, N], FP16, name="vA")
            vB = wp.tile([64, N], FP16, name="vB")

            state = {}

            def body(it=None):
                pGqk = ps.tile([96, 2 * C], F32, name="pGqk", tag="pGqk")
                pGqq = ps.tile([96, 2 * C], F32, name="pGqq", tag="pGqq")
                pGkk = ps.tile([96, 2 * C], F32, name="pGkk", tag="pGkk")
                stage = {}   # (kind, m) -> guard tile
                kTs = {}     # m -> kT group tile
                qTs = {}     # m -> qT group tile

                def new_stage(kind, m):
                    # kind: v0 [128]fp16, v1 [64]fp16, k1 [128(64:128)]fp8,
                    #       k2 [128]fp8
                    shp, dt = {
                        "v0": ([128, GL], FP16), "v1": ([64, GL], FP16),
                        "k1": ([128, GL], FP8), "k2": ([128, GL], FP8),
                    }[kind]
                    t = dwp.tile(shp, dt, name=f"st_{kind}", tag=f"st_{kind}",
                                 bufs=2)
                    stage[(kind, m)] = t
                    p0 = 64 if kind == "k1" else 0
                    pn = shp[0]
                    ms = nc.gpsimd if USE_POOL else nc.vector
                    sl = t[p0:pn, :]
                    # leading pair {0,1} plus the 18 inter-row guard pairs
                    # {130,131,...} in one strided memset (2+W == RS)
                    gap = bass.AP(tensor=sl.tensor, offset=sl.offset,
                                  ap=[sl.ap[0], [RS, 19], [1, 2]])
                    ms.memset(gap, 0.0)
                    if m == 0:
                        ms.memset(t[p0:pn, 2:2 + RS], 0.0)
                    if m == NG - 1:
                        ms.memset(t[p0:pn, 2 + 17 * RS:2 + 18 * RS], 0.0)
                    return t

                def get_stage(kind, m):
                    t = stage.get((kind, m))
                    if t is None:
                        t = new_stage(kind, m)
                    return t

                # ---------------- q conv (fp8 DR) ----------------
                def qconv_tile(t):
                    g = (RT * t) // GR
                    r0 = RT * t
                    if r0 % GR == 0:
                        ty = []
                        tA = io.tile([128, 2 * 18 * WP], FP8, name="y8a",
                                     tag="y8a", bufs=2)
                        nc.sync.dma_start(tA, d_y[g * 288:g * 288 + 128, :])
                        ty.append(tA)
                        tB = io.tile([96, 2 * 18 * WP], FP8, name="y8b",
                                     tag="y8b", bufs=1)
                        nc.sync.dma_start(
                            tB, d_y[g * 288 + 128:g * 288 + 224, :])
                        ty.append(tB)
                        tC = io.tile([64, 2 * 18 * WP], FP8, name="y8c",
                                     tag="y8c", bufs=1)
                        nc.sync.dma_start(
                            tC, d_y[g * 288 + 224:g * 288 + 288, :])
                        ty.append(tC)
                        state["y"] = ty
                    ty = state["y"]
                    if r0 % 16 == 0:
                        qTs[g] = qtp.tile([128, GR * C], FP8, name="qTg",
                                          tag="qTg", bufs=2)
                    t_qT = qTs[g]
                    ptr = ps.tile([128, RT * C], FP16, name="ptrq",
                                  tag="pc", bufs=3)
                    for o0, on in CC:
                        pq = ps.tile([96, 512], F32, name="pq", tag="pq",
                                     bufs=2)
                        nmm = 0
                        for dxi in range(3):
                            for si in range(3):
                                src = ty[si]
                                w8 = t_wq8[si]
                                off = (r0 - 16 * g + 1) * WP + 2 + (dxi - 1)
                                rhs = bass.AP(
                                    tensor=src.tensor, offset=src.offset + off,
                                    ap=[src.ap[0], [18 * WP, 2], [WP, RT],
                                        [1, W]])
                                lhsT = bass.AP(
                                    tensor=w8.tensor,
                                    offset=w8.offset + dxi * C + o0,
                                    ap=[w8.ap[0], [3 * C, 2], [1, on]])
                                nc.tensor.matmul(
                                    pq, lhsT, rhs,
                                    start=(nmm == 0), stop=(nmm == 8),
                                    perf_mode=DR)
                                nmm += 1
                        qs = io.tile([on, 512], FP16, name=f"qs{o0}",
                                     tag="qs", bufs=2)
                        nc.scalar.copy(qs, pq)
                        for j in range(RT):
                            nc.tensor.transpose(
                                ptr[:, j * C + o0:j * C + o0 + on],
                                qs[:, j * 128:(j + 1) * 128],
                                id16[0:96, 0:96])
                    dst = bass.AP(
                        tensor=t_qT.tensor,
                        offset=t_qT.offset + (r0 % 16) * C,
                        ap=[t_qT.ap[0], [C, RT], [1, C]])
                    nc.scalar.copy(dst, ptr)

                # ---------------- kv conv + k-dw ----------------
                def kvconv_tile(t):
                    r0 = RT * t
                    m = t // 4
                    lo = r0 - 16 * m + 1
                    if t % 2 == 0:
                        txp = {}
                        for c0, cn in CC:
                            txp[c0] = io.tile([cn, 1024], FP16,
                                              name=f"xg{c0}", tag=f"xg{c0}",
                                              bufs=2)
                            nc.sync.dma_start(
                                txp[c0],
                                d_x[(t // 2) * C + c0:(t // 2) * C + c0 + cn,
                                    :])
                        state["x"] = txp
                    xo = (t % 2) * 512
                    tx = {c0: state["x"][c0][:, xo:xo + 512] for c0, _ in CC}

                    def evict(pkv, ct):
                        # eviction target(s) for conv chunk ct at rows
                        # [lo, lo+RT) of group m (+ overlap rows)
                        def emit(dstk, psrc, p0, pn):
                            kt = get_stage(dstk, m)
                            sl = kt[p0:pn, :]
                            dst = bass.AP(
                                tensor=sl.tensor,
                                offset=sl.offset + 2 + lo * RS,
                                ap=[sl.ap[0], [RS, RT], [1, W]])
                            nc.scalar.copy(dst, psrc)
                            if r0 % 16 == 0 and m >= 1:
                                pv = stage[(dstk, m - 1)][p0:pn, :]
                                d2 = bass.AP(
                                    tensor=pv.tensor,
                                    offset=pv.offset + 2 + 17 * RS,
                                    ap=[pv.ap[0], [1, W]])
                                nc.scalar.copy(d2, psrc[:, 0:W])
                            if (r0 + 3) % 16 == 15 and m + 1 <= NG - 1:
                                nx = get_stage(dstk, m + 1)[p0:pn, :]
                                d2 = bass.AP(
                                    tensor=nx.tensor,
                                    offset=nx.offset + 2,
                                    ap=[nx.ap[0], [1, W]])
                                nc.scalar.copy(d2, psrc[:, 3 * W:4 * W])
                        if ct == 0:
                            emit("v0", pkv, 0, 128)
                        elif ct == 1:
                            emit("v1", pkv[0:64, :], 0, 64)
                            emit("k1", pkv[64:128, :], 64, 128)
                        else:
                            emit("k2", pkv, 0, 128)

                    for ct in range(3):
                        pkv = ps.tile([128, 512], F32, name="pkv", tag="pc",
                                      bufs=3)
                        for i, (c0, cn) in enumerate(CC):
                            nc.tensor.matmul(
                                pkv, t_wkv[c0][:, ct * 128:(ct + 1) * 128],
                                tx[c0],
                                start=(i == 0), stop=(i == 1))
                        evict(pkv, ct)

                def kdw_tile(t):
                    # fp8 DR diag dw for k chunks, rows r0..r0+3 of group m
                    r0 = RT * t
                    m = t // 4
                    lo = r0 - 16 * m
                    outs = []
                    for kind, wdg, p0, pn in (("k1", t_wdk1, 64, 128),
                                              ("k2", t_wdk2, 0, 128)):
                        src = get_stage(kind, m)[p0:pn, :]
                        kn = pn - p0
                        wsl = wdg[p0:pn, :]
                        pk = ps.tile([128, 512], F32, name=f"pkd{kind}",
                                     tag="pc", bufs=3)
                        for pj in range(5):
                            ta, tb = KD_PAIRS[pj]
                            dlt = (tap_off(tb) - tap_off(ta)) \
                                if tb is not None else 2
                            rhs = bass.AP(
                                tensor=src.tensor,
                                offset=src.offset + tap_off(ta) + lo * RS,
                                ap=[src.ap[0], [dlt, 2], [RS, RT], [1, W]])
                            lhsT = bass.AP(
                                tensor=wsl.tensor,
                                offset=wsl.offset + 2 * pj * kn,
                                ap=[wsl.ap[0], [kn, 2], [1, kn]])
                            nc.tensor.matmul(
                                pk[0:kn, :], lhsT, rhs, start=(pj == 0),
                                stop=(pj == 4), perf_mode=DR)
                        st = io.tile([128, 512], FP16, name=f"kd{kind}",
                                     tag=f"kd{kind}", bufs=2)
                        nc.scalar.copy(st[0:kn, :], pk[0:kn, :])
                        outs.append(st)
                    # transpose k rows [ch, px] -> kT [px, ch] for this tile
                    if r0 % 16 == 0:
                        kTs[m] = io.tile([128, GR * C], FP8, name="kTg",
                                         tag="kTg", bufs=2)
                    kT_g = kTs[m]
                    ptk = ps.tile([128, RT * C], FP16, name="ptk", tag="pq",
                                  bufs=2)
                    st1, st2 = outs
                    for j in range(RT):
                        nc.tensor.transpose(
                            ptk[:, j * C:j * C + 64],
                            st1[0:64, j * 128:(j + 1) * 128],
                            id16[0:64, 0:64])
                        nc.tensor.transpose(
                            ptk[:, j * C + 64:(j + 1) * C],
                            st2[:, j * 128:(j + 1) * 128], id16)
                    dst = bass.AP(
                        tensor=kT_g.tensor,
                        offset=kT_g.offset + (r0 % 16) * C,
                        ap=[kT_g.ap[0], [C, RT], [1, C]])
                    nc.scalar.copy(dst, ptk)

                # ---------------- v dw (DVE/Pool) ----------------
                def vdw_group(g):
                    for kind, pw, wc0, dstv in (("v0", 128, 0, vA),
                                                ("v1", 64, 16, vB)):
                        src = stage[(kind, g)]

                        def sview(tp):
                            return bass.AP(
                                tensor=src.tensor,
                                offset=src.offset + tap_off(tp),
                                ap=[[src.ap[0][0], pw], [RS, GR], [1, W]])

                        acc = dwp.tile([pw, GR * W], FP16, name=f"va{kind}",
                                       tag=f"va{kind}", bufs=2)
                        nc.vector.tensor_scalar_mul(
                            acc, sview(4), t_wdv[0:pw, wc0 + 4:wc0 + 5])
                        order = [0, 1, 2, 3, 5, 6, 7, 8]
                        for i, tp in enumerate(order):
                            last = (i == len(order) - 1)
                            z = dwp.tile([pw, GR * W], FP16, name="dwz",
                                         tag="dwz", bufs=2)
                            eng = nc.gpsimd if (tp == 7 and USE_POOL) \
                                else nc.vector
                            eng.tensor_scalar_mul(
                                z, sview(tp), t_wdv[0:pw, wc0 + tp:wc0 + tp + 1])
                            if last:
                                eng.tensor_add(
                                    dstv[0:pw, 2048 * g:2048 * (g + 1)],
                                    acc, z)
                            else:
                                eng.tensor_add(acc, acc, z)

                # ---------------- grams ----------------
                def gram_group(g):
                    kT_g = kTs[g]
                    qT_g = qTs[g]
                    for jp in range(GR // 2):
                        r = 16 * g + 2 * jp
                        for ci, (c0, cn) in enumerate(CC):
                            first = (r == 0 and ci == 0)
                            last = (r == H - 2 and ci == 1)
                            lq = bass.AP(
                                tensor=qT_g.tensor,
                                offset=qT_g.offset + (2 * jp) * C + c0,
                                ap=[qT_g.ap[0], [C, 2], [1, cn]])
                            rq_ = bass.AP(
                                tensor=qT_g.tensor,
                                offset=qT_g.offset + (2 * jp) * C,
                                ap=[qT_g.ap[0], [C, 2], [1, C]])
                            lk = bass.AP(
                                tensor=kT_g.tensor,
                                offset=kT_g.offset + (2 * jp) * C + c0,
                                ap=[kT_g.ap[0], [C, 2], [1, cn]])
                            rk = bass.AP(
                                tensor=kT_g.tensor,
                                offset=kT_g.offset + (2 * jp) * C,
                                ap=[kT_g.ap[0], [C, 2], [1, C]])
                            nc.tensor.matmul(
                                pGqk[:, ci * C:(ci + 1) * C], lq, rk,
                                start=first, stop=last, perf_mode=DR)
                            nc.tensor.matmul(
                                pGqq[:, ci * C:(ci + 1) * C], lq, rq_,
                                start=first, stop=last, perf_mode=DR)
                            nc.tensor.matmul(
                                pGkk[:, ci * C:(ci + 1) * C], lk, rk,
                                start=first, stop=last, perf_mode=DR)

                # ---------------- main loop ----------------
                for t in range(NT):
                    if "q" in parts:
                        qconv_tile(t)
                    if "kv" in parts:
                        kvconv_tile(t)
                    if "dw" in parts and t >= 1:
                        kdw_tile(t - 1)
                        if (t - 1) % 4 == 3 and "gram" in parts:
                            gram_group((t - 1) // 4)
                    if "dw" in parts and t % 4 == 1 and t >= 5:
                        vdw_group(t // 4 - 1)
                if "dw" in parts:
                    kdw_tile(NT - 1)
                    if "gram" in parts:
                        gram_group(NG - 1)
                    vdw_group(NG - 1)
                if "attn" not in parts:
                    return

                # ---------------- attention ----------------
                # |q| and |k| from gram diagonals
                rq = {}
                kd = {}
                for ci, (c0, cn) in enumerate(CC):
                    dq = sm.tile([cn, cn], F32, name=f"dq{ci}", tag=f"dq{ci}")
                    nc.vector.tensor_mul(
                        dq[:, 0:cn], pGqq[:, ci * C + c0:ci * C + c0 + cn],
                        id32[0:cn, 0:cn])
                    s = sm.tile([cn, 1], F32, name=f"qn2_{c0}",
                                tag=f"qn2_{c0}")
                    nc.vector.tensor_reduce(s, dq[:, 0:cn],
                                            axis=mybir.AxisListType.X,
                                            op=mybir.AluOpType.add)
                    nc.scalar.sqrt(s, s)
                    nc.vector.reciprocal(s, s)
                    nc.vector.tensor_mul(s, s, t_tv[c0][:, 0:1])
                    rq[c0] = s
                    dk = sm.tile([cn, cn], F32, name=f"dk{ci}", tag=f"dk{ci}")
                    nc.vector.tensor_mul(
                        dk[:, 0:cn], pGkk[:, ci * C + c0:ci * C + c0 + cn],
                        id32[0:cn, 0:cn])
                    s2 = sm.tile([cn, 1], F32, name=f"kn2_{c0}",
                                 tag=f"kn2_{c0}")
                    nc.vector.tensor_reduce(s2, dk[:, 0:cn],
                                            axis=mybir.AxisListType.X,
                                            op=mybir.AluOpType.add)
                    kd[c0] = s2
                pkrow = ps.tile([1, C], F32, name="pkrow", tag="pq", bufs=2)
                nc.tensor.transpose(pkrow[0:1, 0:96], kd[0], id32[0:96, 0:96])
                nc.tensor.transpose(pkrow[0:1, 96:192], kd[96],
                                    id32[0:96, 0:96])
                krow = sm.tile([1, C], F32, name="krow", tag="krow")
                nc.scalar.copy(krow, pkrow)
                nc.scalar.sqrt(krow, krow)
                nc.vector.reciprocal(krow, krow)
                rkb = {}
                for c0, cn in CC:
                    pb = ps.tile([cn, C], F32, name=f"prkb{c0}", tag="pc",
                                 bufs=3)
                    nc.tensor.matmul(pb, ones1[0:1, c0:c0 + cn], krow,
                                     start=True, stop=True)
                    sb_ = sm.tile([cn, C], F32, name=f"rkb{c0}",
                                  tag=f"rkb{c0}")
                    nc.scalar.copy(sb_, pb)
                    rkb[c0] = sb_
                A = {}
                for ci, (c0, cn) in enumerate(CC):
                    s = sm.tile([cn, C], F32, name=f"S{c0}", tag=f"S{c0}")
                    nc.vector.tensor_scalar_mul(
                        s, pGqk[:, ci * C:(ci + 1) * C], rq[c0])
                    nc.vector.tensor_mul(s, s, rkb[c0])
                    nc.vector.tensor_add(s, s, t_msk[c0])
                    m = sm.tile([cn, 1], F32, name=f"m{c0}", tag=f"m{c0}")
                    a = sm.tile([cn, C], FP16, name=f"A{c0}", tag=f"A{c0}")
                    z = sm.tile([cn, 1], F32, name=f"z{c0}", tag=f"z{c0}")
                    nc.vector.tensor_reduce(
                        m, s, axis=mybir.AxisListType.X,
                        op=mybir.AluOpType.max)
                    nc.vector.tensor_scalar_mul(m, m, -1.0)
                    nc.scalar.activation(
                        a, s, mybir.ActivationFunctionType.Exp,
                        bias=m, scale=1.0, accum_out=z)
                    nc.vector.reciprocal(z, z)
                    nc.vector.tensor_scalar_mul(a, a, z)
                    A[c0] = a
                CT = {}
                for d0, dn in CC:
                    pc_ = ps.tile([dn, C], F32, name=f"pCT{d0}", tag="pc",
                                  bufs=3)
                    for i, (c0, cn) in enumerate(CC):
                        nc.tensor.matmul(
                            pc_, A[c0][:, d0:d0 + dn], t_wp_[c0],
                            start=(i == 0), stop=(i == 1))
                    ct_ = sm.tile([dn, C], FP16, name=f"CT{d0}",
                                  tag=f"CT{d0}")
                    nc.scalar.copy(ct_, pc_)
                    CT[d0] = ct_
                # shuffle CT rows into CTa (v 0:128) / CTb (v 128:192)
                CTa = sm.tile([128, C], FP16, name="CTa", tag="CTa")
                CTb = sm.tile([64, C], FP16, name="CTb", tag="CTb")
                nc.sync.dma_start(CTa[0:96, :], CT[0])
                nc.sync.dma_start(CTa[96:128, :], CT[96][0:32, :])
                nc.sync.dma_start(CTb[0:64, :], CT[96][32:96, :])

                # ---------------- final conv ----------------
                FTAGS = [("pq", 2), ("pGqk", 1), ("pGqq", 1), ("pGkk", 1),
                         ("pc", 3)]
                fidx = 0
                osts = {}
                for t in range(NT):
                    for o0, on in CC:
                        ftag, fbufs = FTAGS[fidx % len(FTAGS)]
                        fidx += 1
                        pf = ps.tile([on, 512], F32, name=f"pf{o0}",
                                     tag=ftag, bufs=fbufs)
                        nc.tensor.matmul(
                            pf, CTa[:, o0:o0 + on],
                            vA[:, 512 * t:512 * (t + 1)],
                            start=True, stop=False)
                        nc.tensor.matmul(
                            pf, CTb[:, o0:o0 + on],
                            vB[:, 512 * t:512 * (t + 1)],
                            start=False, stop=True)
                        if t % 2 == 0:
                            osts[o0] = io.tile([on, 1024], F32,
                                               name=f"ost{o0}",
                                               tag=f"ost{o0}", bufs=2)
                        ost = osts[o0]
                        if o0 == 0:
                            nc.scalar.copy(ost[:, (t % 2) * 512:
                                               (t % 2) * 512 + 512], pf)
                        else:
                            nc.vector.tensor_copy(
                                ost[:, (t % 2) * 512:(t % 2) * 512 + 512], pf)
                        if t % 2 == 1:
                            eng = nc.scalar if o0 == 0 else nc.sync
                            eng.dma_start(
                                d_out[(t // 2) * C + o0:
                                      (t // 2) * C + o0 + on, :], ost)

            if use_for_i and repeat > 1:
                with tc.For_i(0, repeat, 1) as iv:
                    body(iv)
            else:
                for it in range(repeat):
                    body(it)

            o16 = sm.tile([128, 16], F32, name="o16", tag="o16")
            nc.vector.tensor_copy(o16, tki)
            nc.sync.dma_start(d_tok[:, :], o16)

    nc.compile()
    return nc


# ---------------------------------------------------------------------------
# host-side packing
# ---------------------------------------------------------------------------

def prep_weights(kv_w, kv_dw_w, q_w, q_dw_w, proj_w, temperature):
    kv_w = np.asarray(kv_w, np.float32).reshape(2 * C, C)
    kv_dw_w = np.asarray(kv_dw_w, np.float32).reshape(2 * C, 9)
    q_w = np.asarray(q_w, np.float32).reshape(C, C)
    q_dw_w = np.asarray(q_dw_w, np.float32).reshape(C, C, 9)
    proj_w = np.asarray(proj_w, np.float32).reshape(C, C)
    temperature = np.asarray(temperature, np.float32).reshape(HEADS)

    wq = np.einsum('oct,ci->oit', q_dw_w, q_w)               # [o, i, tap]
    wq3 = np.zeros((3 * C, 3 * C), np.float32)
    for dy in range(3):
        for dx in range(3):
            tap = dy * 3 + dx
            wq3[dy * C:(dy + 1) * C, dx * C:(dx + 1) * C] = wq[:, :, tap].T
    wq_lhsT = (wq3 * WQ_SCALE).astype(ml_dtypes.float8_e4m3)

    # kv conv1x1 with output order [v(192); k(192)]
    kv_vk = np.concatenate([kv_w[C:2 * C], kv_w[0:C]], 0)     # [v; k]
    wkv_lhsT = np.ascontiguousarray(kv_vk.T).astype(np.float16)

    dw_k = kv_dw_w[0:C]      # k-channel dw filters [C, 9]
    dw_v = kv_dw_w[C:2 * C]  # v-channel dw filters [C, 9]
    # v dw scalars: ct0 = v 0:128, ct1v = v 128:192
    wdv = np.zeros((2 * 128, 16), np.float32)
    wdv[0:128, :9] = dw_v[0:128]
    wdv[128:128 + 64, :9] = dw_v[128:192]
    # k dw diag fp8 (scaled): ct1k = k 0:64 (parts 64:128), ct2 = k 64:192
    # layout [part, plane(10), col]: plane 2j/2j+1 = taps of KD_PAIRS[j]
    plane_taps = []
    for ta, tb in KD_PAIRS:
        plane_taps.extend([ta, tb])
    wdk1_t = np.zeros((128, 10, 64), np.float32)
    wdk2_t = np.zeros((128, 10, 128), np.float32)
    for pl, tap in enumerate(plane_taps):
        if tap is None:
            continue
        for i in range(64):
            wdk1_t[64 + i, pl, i] = dw_k[i, tap] * WK_SCALE
        for p in range(128):
            wdk2_t[p, pl, p] = dw_k[64 + p, tap] * WK_SCALE
    wdk1_t = wdk1_t.reshape(128, 10 * 64).astype(ml_dtypes.float8_e4m3)
    wdk2_t = wdk2_t.reshape(128, 10 * 128).astype(ml_dtypes.float8_e4m3)

    wproj_T = np.ascontiguousarray(proj_w.T).astype(np.float16)
    tvec = np.zeros((C, 16), np.float32)
    tvec[:, 0] = np.repeat(temperature, CH)
    smask = np.full((C, C), -60000.0, np.float32)
    for h in range(HEADS):
        smask[h * CH:(h + 1) * CH, h * CH:(h + 1) * CH] = 0.0
    return (wq_lhsT, wkv_lhsT, wdv, wdk1_t, wdk2_t, wproj_T, tvec, smask)


def prep_image(xi, yi):
    x16 = np.asarray(xi, np.float32).reshape(C, NT // 2, 1024).astype(
        np.float16)
    x16 = np.ascontiguousarray(x16.transpose(1, 0, 2)).reshape(
        (NT // 2) * C, 1024)
    yp = np.zeros((C, HP + 2, WP), ml_dtypes.float8_e4m3)
    yp[:, 2:2 + H, 2:2 + W] = np.asarray(yi, np.float32).astype(
        ml_dtypes.float8_e4m3)
    # pack per-group DR subtiles: rows g*288+[A(128)|B(96)|C(64)],
    # cols [kt, row(18), WP]
    ypk = np.zeros((NG * 288, 2, 18, WP), ml_dtypes.float8_e4m3)
    for g in range(NG):
        r = 16 * g
        base = g * 288
        ypk[base:base + 128, 0] = yp[0:128, r:r + 18]
        ypk[base:base + 64, 1] = yp[128:192, r:r + 18]
        ypk[base + 64:base + 128, 1] = yp[0:64, r + 1:r + 19]
        ypk[base + 128:base + 224, 0] = yp[64:160, r + 1:r + 19]
        ypk[base + 128:base + 160, 1] = yp[160:192, r + 1:r + 19]
        ypk[base + 160:base + 224, 1] = yp[0:64, r + 2:r + 20]
        ypk[base + 224:base + 288, 0] = yp[64:128, r + 2:r + 20]
        ypk[base + 224:base + 288, 1] = yp[128:192, r + 2:r + 20]
    return x16, ypk.reshape(NG * 288, 2 * 18 * WP)


_CACHE = {}


def make_in_maps(x, y, kv_w, kv_dw_w, q_w, q_dw_w, proj_w, temperature):
    x = np.asarray(x, np.float32)
    y = np.asarray(y, np.float32)
    wq, wkv, wdv, wdk1, wdk2, wpj, tv, smask = prep_weights(
        kv_w, kv_dw_w, q_w, q_dw_w, proj_w, temperature)
    tok = np.zeros((128, 16), np.float32)
    in_maps = []
    for b in range(B):
        x16, yp = prep_image(x[b], y[b])
        in_maps.append({
            "tok_in": tok, "ypad": yp, "x16": x16,
            "wq": wq, "wkv": wkv, "wdv": wdv, "wdk1": wdk1, "wdk2": wdk2,
            "wproj": wpj, "tvec": tv, "smask": smask,
        })
    return in_maps


def kernel(x, y, kv_w, kv_dw_w, q_w, q_dw_w, proj_w, temperature):
    in_maps = make_in_maps(x, y, kv_w, kv_dw_w, q_w, q_dw_w, proj_w,
                           temperature)
    if "nc" not in _CACHE:
        _CACHE["nc"] = build()
    nc = _CACHE["nc"]
    from concourse.bass_utils import run_bass_kernel_spmd
    res = run_bass_kernel_spmd(nc, in_maps, core_ids=list(range(B)))
    out = np.stack([
        res.results[b]["out"].reshape(NT // 2, C, 1024).transpose(1, 0, 2)
        .reshape(C, H, W) for b in range(B)])
    return out.astype(np.float32)
